# revision 101
# baseline (speedup 1.0000x reference)
"""BiMPM matching kernel for Trainium2 (Bass/Tile), 8-core data-parallel.

Strategy: batch B=8 is sharded one element per NeuronCore. Each core runs the
full BiMPM forward for its (L=128, D=512) pair of contexts.

v3 design (TimelineSim ~132.6us vs the 229.2us select-broadcast baseline):
  - The dominant attentive-max tensors (max_j cos[i,j]*c[j,d], both sides)
    are computed without PE or PSUM: the masked fp16 contexts round-trip
    through scratch DRAM once, then one DMA per 4-row "quad" re-reads a row
    with a zero-stride (broadcast) source AP, replicating it across all 128
    partitions straight into SBUF. The per-k cosine multiply is routed per
    quad to whichever engine has headroom - ACT (scaled copy), Pool/GPSIMD
    (tensor_scalar; the only tensor ops walrus accepts on that engine), or
    DVE itself at 4x fp16 for the early quads while the DMA ring warms up.
    DVE then max-accumulates (128,2048) fp16 tiles at 2x on two interleaved
    chains per side. This keeps DVE, ACT, Pool, and the DMA engines all
    ~95-100% busy through the body of the kernel.
  - All "single + 16 multi-perspective" cosine feature blocks use a 17-wide
    weight layout (leading ones column) so one matmul chain yields the s and
    m features contiguously, including the self-norm via the ones column.
  - Softmax normalization for attentive-mean is dropped entirely: cosine
    matching is scale-invariant per row, so a raw masked exp() suffices.
  - Maxpool / full-match / attentive-match work is decomposed into
    generator "tasks" advanced one pipeline stage per scheduler tick, so
    every engine's in-order stream stays dependency-ready.

Self-contained: hardcodes shapes B=8, L1=L2=128, D=512, P=16.
"""

import numpy as np

import concourse.bass as bass
import concourse.mybir as mybir
import concourse.tile as tile
from concourse.bass_utils import run_bass_kernel_spmd
from concourse.vector_clock import ScopedClock

f32 = mybir.dt.float32
f32r = mybir.dt.float32r
f16 = mybir.dt.float16
ALU = mybir.AluOpType
AFT = mybir.ActivationFunctionType
AX = mybir.AxisListType

B, L, D, P = 8, 128, 512, 16
NCH = D // 128  # 4 d-chunks
P17 = P + 1
NEG = -1.0e30
EPS_CNT = 1.0e-8  # matches reference EPS for count clamping
EPS_N = 1.0e-6    # per-factor norm clamp (product >= 1e-12 never binds here)
OFFBIG = 60000.0  # fp16-finite sentinel for attentive-max masking

# ---------------------------------------------------------------------------
# Workarounds: this walrus build accepts only ONE sync-wait per instruction.
# ---------------------------------------------------------------------------

def _drain_and_barrier_split(self, tick_clock, wait_clock):
    drain_inst = self.nc.sync.drain()
    wait_clock.add_sem_waits(
        drain_inst.ins, ScopedClock({None: tick_clock.global_clock})
    )
    si = drain_inst.ins.sync_info
    if si is not None and si.on_wait and len(si.on_wait) > 1:
        extra = list(si.on_wait[1:])
        del si.on_wait[1:]
        for w in extra:
            d2 = self.nc.sync.drain()
            if d2.ins.sync_info is None:
                d2.ins.sync_info = mybir.SyncInfo(on_wait=[], on_update=[])
            d2.ins.sync_info.on_wait.append(w)
    self.nc.all_engine_barrier()
    assert self.sems is not None
    popped = self.nc._tile_sem_poison_stack.pop()
    assert popped is self._sem_poison
    self.nc.clear_and_free_semaphores(list(self.sems.allocated().values()))


tile.TileContext._drain_and_barrier = _drain_and_barrier_split


def _split_multi_waits(nc):
    """Hoist extra sync-waits onto injected same-engine Drains placed before
    the owning instruction (serial on one engine == wait-all)."""
    n = 0
    for fn in nc.m.functions:
        for blk in fn.blocks:
            new = []
            for ins in blk.instructions:
                si = ins.sync_info
                if si is not None and si.on_wait and len(si.on_wait) > 1:
                    extra = list(si.on_wait[:-1])
                    keep = [si.on_wait[-1]]
                    for w in extra:
                        new.append(
                            mybir.InstDrain(
                                name=f"waitsplit-{n}",
                                engine=ins.engine,
                                is_reset_sema=False,
                                sync_info=mybir.SyncInfo(on_wait=[w], on_update=[]),
                            )
                        )
                        n += 1
                    si.on_wait = keep
                new.append(ins)
            blk.instructions = new
    return n


# ---------------------------------------------------------------------------
# Kernel emission
# ---------------------------------------------------------------------------

def CH(k):
    return slice(k * 128, (k + 1) * 128)


def C17(k):
    return slice(k * P17, (k + 1) * P17)


def _emit(nc: bass.Bass):
    ctx1_d = nc.dram_tensor("context_1", [L, D], f32, kind="ExternalInput")
    m1_d = nc.dram_tensor("mask_1", [1, L], f32, kind="ExternalInput")
    ctx2_d = nc.dram_tensor("context_2", [L, D], f32, kind="ExternalInput")
    m2_d = nc.dram_tensor("mask_2", [1, L], f32, kind="ExternalInput")
    wff_d = nc.dram_tensor("w_full_fwd", [P, D], f32, kind="ExternalInput")
    wbw_d = nc.dram_tensor("w_full_bwd", [P, D], f32, kind="ExternalInput")
    wmp_d = nc.dram_tensor("w_maxpool", [P, D], f32, kind="ExternalInput")
    wat_d = nc.dram_tensor("w_att", [P, D], f32, kind="ExternalInput")
    wma_d = nc.dram_tensor("w_max_att", [P, D], f32, kind="ExternalInput")
    idn_d = nc.dram_tensor("idn", [128, 128], f32, kind="ExternalInput")
    out_d = nc.dram_tensor("out", [L, 204], f32, kind="ExternalOutput")

    c1M_d = nc.dram_tensor("c1M_scr", [L, D], f16, kind="Internal")
    c2M_d = nc.dram_tensor("c2M_scr", [L, D], f16, kind="Internal")

    with tile.TileContext(nc) as tc:
        with tc.tile_pool(name="sb", bufs=1) as sb, \
             tc.tile_pool(name="sc", bufs=2) as sc, \
             tc.tile_pool(name="psX", bufs=6, space="PSUM") as psX:

            def xt(shape=None, name="x"):
                return psX.tile(shape or [128, 512], f32, tag="x", name=name,
                                padded_shape=[128, 512])

            def scr512():
                return sc.tile([128, 512], f32, tag="scr512", name="scr512")

            # ---------------- constants + inputs ----------------
            idn = sb.tile([128, 128], f32, tag="idn")
            nc.sync.dma_start(idn[:], idn_d[:])
            ones1 = sb.tile([1, 128], f32, tag="ones1")
            nc.vector.memset(ones1[:], 1.0)
            one11 = sb.tile([1, 1], f32, tag="one11")
            nc.vector.memset(one11[:], 1.0)
            # sqrt-bias clamp: sqrt(x + 1e-12) == max(sqrt(x), 1e-6)
            epsb = sb.tile([128, 1], f32, tag="epsb")
            nc.vector.memset(epsb[:], 1.0e-12)

            ctx1 = sb.tile([128, 512], f32, tag="ctx1")
            nc.sync.dma_start(ctx1[:], ctx1_d[:])
            ctx2 = sb.tile([128, 512], f32, tag="ctx2")
            nc.sync.dma_start(ctx2[:], ctx2_d[:])
            m1row = sb.tile([1, 128], f32, tag="m1row")
            nc.sync.dma_start(m1row[:], m1_d[:])
            m2row = sb.tile([1, 128], f32, tag="m2row")
            nc.sync.dma_start(m2row[:], m2_d[:])
            wdr = {}
            for wname, wd in (("ff", wff_d), ("bw", wbw_d), ("mp", wmp_d),
                              ("at", wat_d), ("ma", wma_d)):
                wt = sb.tile([P, 512], f32, tag=f"w_{wname}", name=f"w_{wname}")
                nc.sync.dma_start(wt[:], wd[:])
                wdr[wname] = wt

            out12 = sb.tile([128, 204], f32, tag="out12")

            # ---------------- masks / columns ----------------
            def row_to_col(row, n=128):
                ps = xt(name="r2c")
                nc.tensor.matmul(ps[:n, 0:1], lhsT=row[:, 0:n], rhs=one11[:],
                                 start=True, stop=True)
                col = sb.tile([n, 1], f32, tag=f"col{nc.next_id()}", name="col")
                nc.vector.tensor_copy(col[:], ps[:n, 0:1])
                return col

            m1col = row_to_col(m1row)
            m2col = row_to_col(m2row)

            def ts_new(shape, tag, in0, s1, s2, op0, op1=None):
                t = sb.tile(shape, f32, tag=tag, name=tag)
                if op1 is None:
                    nc.vector.tensor_scalar(t[:], in0[:], s1, None, op0)
                else:
                    nc.vector.tensor_scalar(t[:], in0[:], s1, s2, op0, op1)
                return t

            # softmax bias (-1e30 at invalid rows, f32 domain)
            offm1col = ts_new([128, 1], "offm1col", m1col, -1.0, 1.0e30, ALU.add, ALU.mult)
            offm2col = ts_new([128, 1], "offm2col", m2col, -1.0, 1.0e30, ALU.add, ALU.mult)
            # att-max sentinels (fp16-finite)
            offb1col = ts_new([128, 1], "offb1col", m1col, -1.0, OFFBIG, ALU.add, ALU.mult)
            offb2col = ts_new([128, 1], "offb2col", m2col, -1.0, OFFBIG, ALU.add, ALU.mult)
            # +1 at invalid columns (for the cosM shift)
            invm1row = ts_new([1, 128], "invm1row", m1row, -1.0, 1.0, ALU.mult, ALU.add)
            invm2row = ts_new([1, 128], "invm2row", m2row, -1.0, 1.0, ALU.mult, ALU.add)
            # -1e30 at invalid columns (maxpool fills, f32 domain)
            offm1row = ts_new([1, 128], "offm1row", m1row, -1.0, 1.0e30, ALU.add, ALU.mult)
            offm2row = ts_new([1, 128], "offm2row", m2row, -1.0, 1.0e30, ALU.add, ALU.mult)

            # counts: rcnt = 1/max(sum(mask), EPS)
            def rcnt_of(mrow, tag):
                s = sb.tile([1, 1], f32, tag=f"cnt_{tag}", name="cnt")
                nc.vector.tensor_reduce(s[:], mrow[:], AX.X, ALU.add)
                sc_ = sb.tile([1, 1], f32, tag=f"cntc_{tag}", name="cntc")
                nc.vector.tensor_scalar(sc_[:], s[:], EPS_CNT, None, ALU.max)
                r = sb.tile([1, 1], f32, tag=f"rcnt_{tag}", name="rcnt")
                nc.vector.reciprocal(r[:], sc_[:])
                return r

            rcnt1 = rcnt_of(m1row, "1")
            rcnt2 = rcnt_of(m2row, "2")
            m1rowS = ts_new([1, 128], "m1rowS", m1row, rcnt1[:], None, ALU.mult)
            m2rowS = ts_new([1, 128], "m2rowS", m2row, rcnt2[:], None, ALU.mult)
            m1sd = row_to_col(m1rowS)  # mask/cnt column, for PE mean-reduces
            m2sd = row_to_col(m2rowS)

            # broadcast rows across partitions (PE outer product)
            def bcast_row(row, tag, act=False):
                ps = xt(name="bcr")
                nc.tensor.matmul(ps[:, 0:128], lhsT=ones1[:], rhs=row[:],
                                 start=True, stop=True)
                t = sb.tile([128, 128], f32, tag=tag, name=tag)
                if act:
                    nc.scalar.copy(t[:], ps[:, 0:128])
                else:
                    nc.vector.tensor_copy(t[:], ps[:, 0:128])
                return t

            bcOff1 = bcast_row(offm1row, "bcOff1")
            bcOff2 = bcast_row(offm2row, "bcOff2")

            # ---------------- norms + normalized contexts ----------------
            def normalize(cx, mcol_, tag):
                nsq = sb.tile([128, 1], f32, tag=f"nsq_{tag}", name="nsq")
                nc.scalar.activation(scr512()[:], cx[:], AFT.Square, accum_out=nsq[:])
                nn_ = sb.tile([128, 1], f32, tag=f"nn_{tag}", name="nn")
                nc.scalar.activation(nn_[:], nsq[:], AFT.Sqrt, bias=epsb[:])
                rn = sb.tile([128, 1], f32, tag=f"rn_{tag}", name="rn")
                nc.vector.reciprocal(rn[:], nn_[:])
                # fold the row mask into the normalization scale
                rnm = sb.tile([128, 1], f32, tag=f"rnm_{tag}", name="rnm")
                nc.vector.tensor_tensor(rnm[:], rn[:], mcol_[:], ALU.mult)
                cn = sb.tile([128, 512], f32, tag=f"cn_{tag}", name="cn")
                nc.scalar.activation(cn[:], cx[:], AFT.Copy, scale=rnm[:])
                return cn

            cn1 = normalize(ctx1, m1col, "1")
            cn2 = normalize(ctx2, m2col, "2")

            # transposed normalized contexts: cT (f32 for cosine) + fp16 copy
            def transpose_pair(src, tag):
                ps = xt(name=f"T_{tag}")
                for k in range(NCH):
                    nc.tensor.transpose(ps[:, CH(k)], src[:, CH(k)], idn[:])
                t32 = sb.tile([128, 512], f32, tag=f"{tag}32", name=f"{tag}32")
                nc.scalar.copy(t32[:], ps[:])
                t16 = sb.tile([128, 512], f16, tag=f"{tag}16", name=f"{tag}16")
                nc.vector.tensor_copy(t16[:], ps[:])
                return t32, t16

            c1T, c1TL = transpose_pair(cn1, "c1T")
            c2T, c2TL = transpose_pair(cn2, "c2T")
            c1sqT = sb.tile([128, 512], f16, tag="c1sqT")
            nc.scalar.square(c1sqT[:], c1TL[:])
            c2sqT = sb.tile([128, 512], f16, tag="c2sqT")
            nc.scalar.square(c2sqT[:], c2TL[:])

            # masked offsets for the att-max inputs (Pool add, fp16 out),
            # then staged to scratch DRAM for the broadcast loop DMAs
            c1M = sb.tile([128, 512], f16, tag="c1M")
            nc.gpsimd.tensor_scalar(c1M[:], ctx1[:], offb1col[:], None, ALU.add)
            c2M = sb.tile([128, 512], f16, tag="c2M")
            nc.gpsimd.tensor_scalar(c2M[:], ctx2[:], offb2col[:], None, ALU.add)
            nc.sync.dma_start(c1M_d[:], c1M[:])
            nc.sync.dma_start(c2M_d[:], c2M[:])

            # ---------------- cosine ----------------
            cos_ps = xt(name="cos_ps")
            for k in range(NCH):
                nc.tensor.matmul(cos_ps[:, 0:128], lhsT=c1T[:, CH(k)],
                                 rhs=c2T[:, CH(k)],
                                 start=(k == 0), stop=(k == NCH - 1))
            cos = sb.tile([128, 128], f32, tag="cos")
            nc.vector.tensor_copy(cos[:], cos_ps[:, 0:128])
            # bake the +1-at-invalid-j shift into the PSUM, then copy (scales)
            nc.tensor.matmul(cos_ps[:, 0:128], lhsT=ones1[:], rhs=invm2row[:],
                             start=False, stop=True, skip_group_check=True)
            cosM = sb.tile([128, 128], f32, tag="cosM")
            nc.vector.tensor_copy(cosM[:], cos_ps[:, 0:128])

            cosT_ps = xt(name="cosT_ps")
            nc.tensor.transpose(cosT_ps[:, 0:128], cos[:], idn[:])
            cosT = sb.tile([128, 128], f32, tag="cosT")
            nc.vector.tensor_copy(cosT[:], cosT_ps[:, 0:128])
            nc.tensor.matmul(cosT_ps[:, 0:128], lhsT=ones1[:], rhs=invm1row[:],
                             start=False, stop=True, skip_group_check=True)
            cosMT = sb.tile([128, 128], f32, tag="cosMT")
            nc.vector.tensor_copy(cosMT[:], cosT_ps[:, 0:128])
            idnL = sb.tile([128, 128], f16, tag="idnL")
            nc.gpsimd.tensor_copy(idnL[:], idn[:])

            # ---------------- cos_max / cos_mean (out cols 0,1 / 102,103) ----
            def cos_features():
                scrs = []
                for (csrc, cTsrc, bcOff, msd, base) in (
                        (cos, cosT, bcOff2, m2sd, 0),
                        (cosT, cos, bcOff1, m1sd, 102)):
                    t = sc.tile([128, 128], f32, tag="cfscr", name="cfscr")
                    nc.vector.tensor_tensor(t[:], csrc[:], bcOff[:], ALU.add)
                    mps = xt(name="cmean")
                    nc.tensor.matmul(mps[:, 0:1], lhsT=cTsrc[:], rhs=msd[:],
                                     start=True, stop=True)
                    scrs.append((t, mps, base))
                yield
                for t, mps, base in scrs:
                    nc.vector.tensor_reduce(out12[:, base:base + 1], t[:],
                                            AX.X, ALU.max)
                    nc.vector.tensor_copy(out12[:, base + 1:base + 2],
                                          mps[:, 0:1])

            # ---------------- per-weight prep: wsqT17 + rnp17 ----------------
            # wsqT17: (128, 68) fp16; chunk k cols [17k]=ones, [17k+1..17k+16]=
            # (w^2 chunk k Transposed). rnp17: (128,17) with col0 = 1 (self
            # rows are unit-norm), cols 1..16 = 1/||w_p o cn||.
            wsqT17 = {}
            rnp17 = {"1": {}, "2": {}}

            def prep_w(wname):
                wt = wdr[wname]
                wT = sb.tile([128, 68], f16, tag=f"wsqT_{wname}", name="wsqT")
                nc.gpsimd.memset(wT[:], 1.0)
                wsq = sc.tile([P, 512], f32, tag="wsq", name="wsq", bufs=3)
                nc.scalar.square(wsq[:], wt[:])
                yield
                psW = xt(name="psW")
                for k in range(NCH):
                    nc.tensor.transpose(psW[:, 16 * k:16 * (k + 1)],
                                        wsq[:, CH(k)], idn[0:P, 0:P])
                yield
                for k in range(NCH):
                    nc.vector.tensor_copy(wT[:, 17 * k + 1:17 * (k + 1)],
                                          psW[:, 16 * k:16 * (k + 1)])
                wsqT17[wname] = wT
                if wname == "mp":
                    w32 = sb.tile([128, 64], f32, tag="wsqT32mp", name="wsqT32")
                    nc.vector.tensor_copy(w32[:], psW[:, 0:64])
                    wsqT17["mp32"] = w32

            def prep_rnp(wname, side):
                csqT = c1sqT if side == "1" else c2sqT
                ps = xt(name="psnp")
                for k in range(NCH):
                    nc.tensor.matmul(ps[:, 0:P17], lhsT=csqT[:, CH(k)],
                                     rhs=wsqT17[wname][:, C17(k)],
                                     start=(k == 0), stop=(k == NCH - 1))
                yield
                sq = sb.tile([128, P17], f32, tag=f"npsq_{wname}{side}", name="npsq")
                nc.scalar.activation(sq[:], ps[:, 0:P17], AFT.Sqrt,
                                     bias=epsb[:])
                yield
                r = sb.tile([128, P17], f32, tag=f"rnp_{wname}{side}", name="rnp")
                nc.vector.reciprocal(r[:], sq[:])
                rnp17[side][wname] = r

            # ---------------- attentive-max loop pieces ----------------
            # Per side and k-quad: one DMA broadcasts rows 4t..4t+3 of the
            # DRAM-staged cM to all 128 partitions (SBUF fp16). The per-k
            # cosine multiply runs on ACT (scaled copy) or Pool (tensor
            # scalar); DVE only max-accumulates (fp16 2x), on two chains per
            # side. No PE or PSUM in the loop.
            accB = {"2": [sb.tile([128, 4, 512], f16, tag=f"acc2{c}",
                                  name="acc") for c in (0, 1)],
                    "1": [sb.tile([128, 4, 512], f16, tag=f"acc1{c}",
                                  name="acc") for c in (0, 1)]}
            first_b = {"2": [True, True], "1": [True, True]}

            NPEQ = 0  # early quads per side routed via PE/PSUM (DMA is busy
            # with input loads then; PE is otherwise idle)

            def loop_produce(side, q):
                """Stage k = 4q..4q+3 (a 'quad'): broadcast DMA from scratch
                DRAM, or PE select-broadcast into PSUM for the early quads."""
                if q < NPEQ:
                    rhs = c2M if side == "2" else c1M
                    pss = []
                    for u in range(4):
                        ps = xt(name="peq")
                        nc.tensor.matmul(
                            ps[:],
                            lhsT=idnL[:, 4 * q + u:4 * q + u + 1]
                            .broadcast_to([128, 128]),
                            rhs=rhs[:], start=True, stop=True,
                            skip_group_check=True)
                        pss.append(ps)
                    return pss
                src_d = c2M_d if side == "2" else c1M_d
                stg = sc.tile([128, 4, 512], f16, tag="stg", bufs=8,
                              name="stg")
                nc.sync.dma_start(
                    stg[:], src_d[4 * q:4 * q + 4, :].unsqueeze(0)
                    .broadcast_to([128, 4, 512]))
                return stg

            def loop_consume(side, q, stg):
                """Consume one staged quad: 4 scaled mults + one fused max."""
                k0 = 4 * q
                csc = cosM if side == "2" else cosMT
                chain = q % 2
                pe_quad = q < NPEQ
                dve_quad = (not pe_quad) and q < (6 if side == "2" else 5)
                use_pool = (not pe_quad) and (not dve_quad) and (
                    (q % 9 in (1, 3, 5, 7)) if side == "2" else
                    (q % 9 in (0, 2, 4, 6)))
                if first_b[side][chain]:
                    dst = accB[side][chain]
                    first_b[side][chain] = False
                else:
                    dst = sc.tile([128, 4, 512], f16, tag="bch", bufs=12,
                                  name="bch")
                for u in range(4):
                    src = stg[u][:] if pe_quad else stg[:, u, :]
                    if use_pool:
                        nc.gpsimd.tensor_scalar(
                            dst[:, u, :], src,
                            csc[:, k0 + u:k0 + u + 1], None, ALU.mult)
                    elif dve_quad and u < 2:
                        nc.vector.tensor_scalar(
                            dst[:, u, :], src,
                            csc[:, k0 + u:k0 + u + 1], None, ALU.mult)
                    else:
                        nc.scalar.activation(
                            dst[:, u, :], src, AFT.Copy,
                            scale=csc[:, k0 + u:k0 + u + 1])
                if dst is not accB[side][chain]:
                    nc.vector.tensor_tensor(accB[side][chain][:], dst[:],
                                            accB[side][chain][:], ALU.max)

            def loop_finish(side):
                m1 = sb.tile([128, 4, 512], f16, tag=f"axm_{side}", name="axm")
                nc.vector.tensor_tensor(m1[:], accB[side][0][:],
                                        accB[side][1][:], ALU.max)
                m2 = sb.tile([128, 2, 512], f16, tag=f"axn_{side}", name="axn")
                nc.vector.tensor_tensor(m2[:], m1[:, 0:2, :], m1[:, 2:4, :],
                                        ALU.max)
                ax = sb.tile([128, 512], f32, tag=f"ax_{side}", name="ax")
                nc.vector.tensor_tensor(ax[:], m2[:, 0, :], m2[:, 1, :],
                                        ALU.max)
                return ax

            # ---------------- maxpool matching ----------------
            def mp_iter(p):
                rnp1mp = rnp17["1"]["mp"]
                rnp2mp = rnp17["2"]["mp"]
                w32 = wsqT17["mp32"]
                wc = sc.tile([128, 512], f16, tag="wc", bufs=3, name="wc")
                for k in range(NCH):
                    nc.vector.tensor_scalar(
                        wc[:, CH(k)], c1TL[:, CH(k)],
                        w32[:, 16 * k + p:16 * k + p + 1], None, ALU.mult)
                yield
                mp_ps = xt(name="mp_ps")
                for k in range(NCH):
                    nc.tensor.matmul(mp_ps[:, 0:128], lhsT=wc[:, CH(k)],
                                     rhs=c2TL[:, CH(k)],
                                     start=(k == 0), stop=(k == NCH - 1))
                yield
                t1 = sc.tile([128, 128], f32, tag="mv_t1", bufs=3, name="mv_t1")
                if p % 2 == 0:
                    nc.scalar.activation(t1[:], mp_ps[:, 0:128], AFT.Copy,
                                         scale=rnp1mp[:, 1 + p:2 + p])
                else:
                    nc.vector.tensor_scalar(t1[:], mp_ps[:, 0:128],
                                            rnp1mp[:, 1 + p:2 + p], None,
                                            ALU.mult)
                yield
                t1T_ps = xt(name="t1T")
                nc.tensor.transpose(t1T_ps[:, 0:128], t1[:], idn[:])
                # fold the mask-1 fill (along free i) in via a PE accumulate
                nc.tensor.matmul(t1T_ps[:, 0:128], lhsT=ones1[:], rhs=offm1row[:],
                                 start=False, stop=True, skip_group_check=True)
                yield
                npt = sc.tile([128, 128], f32, tag="mv_npt", bufs=3, name="mv_npt")
                if p % 2 == 1:
                    nc.scalar.activation(npt[:], t1T_ps[:, 0:128], AFT.Copy,
                                         scale=rnp2mp[:, 1 + p:2 + p])
                else:
                    nc.vector.tensor_scalar(npt[:], t1T_ps[:, 0:128],
                                            rnp2mp[:, 1 + p:2 + p], None,
                                            ALU.mult)
                yield
                np_ps = xt(name="npT")
                nc.tensor.transpose(np_ps[:, 0:128], npt[:], idn[:])
                nc.tensor.matmul(np_ps[:, 0:128], lhsT=ones1[:], rhs=offm2row[:],
                                 start=False, stop=True, skip_group_check=True)
                # masked means as PE reductions against mask/cnt columns,
                # sharing the np_ps PSUM tile (cols 128,129)
                nc.tensor.matmul(np_ps[:, 128:129], lhsT=npt[:], rhs=m2sd[:],
                                 start=True, stop=True, skip_group_check=True)
                nc.tensor.matmul(np_ps[:, 129:130], lhsT=t1[:], rhs=m1sd[:],
                                 start=True, stop=True, skip_group_check=True)
                yield
                # (i,j) orientation (np_ps, PSUM) reduces over j; (j,i) over i
                nc.vector.tensor_reduce(out12[:, 36 + p:37 + p],
                                        np_ps[:, 0:128], AX.X, ALU.max)
                nc.vector.tensor_reduce(out12[:, 102 + 36 + p:102 + 37 + p],
                                        npt[:], AX.X, ALU.max)
                nc.vector.tensor_copy(out12[:, 52 + p:53 + p], np_ps[:, 128:129])
                nc.vector.tensor_scalar(out12[:, 102 + 52 + p:102 + 53 + p],
                                        np_ps[:, 129:130], rnp2mp[:, 1 + p:2 + p],
                                        None, ALU.mult)

            def mp_fixups():
                # invalid-i rows of the mv1 blocks picked up the transposed
                # mask-1 fill term; reference value there is exactly 0, and
                # (-huge) * 0 == -0, so a mask multiply restores it.
                nc.gpsimd.tensor_scalar(out12[:, 36:68], out12[:, 36:68],
                                        m1col[:], None, ALU.mult)

            # ---------------- full matching (last/first rows) ----------------
            def onehot_last(mrow, tag):
                oh = sb.tile([1, 128], f32, tag=f"oh_{tag}", name="oh")
                nc.vector.tensor_sub(oh[:, 0:127], mrow[:, 0:127], mrow[:, 1:128])
                nc.vector.tensor_copy(oh[:, 127:128], mrow[:, 127:128])
                return oh

            def extract_row(coltile, src, tag):
                ps = xt(name="exr")
                nc.tensor.matmul(ps[0:1, :], lhsT=coltile[:], rhs=src[:],
                                 start=True, stop=True)
                t = sb.tile([1, 512], f32, tag=f"row_{tag}", name="rowx")
                nc.vector.tensor_copy(t[:], ps[0:1, :])
                return t

            def row_match(rowsrc, wname, side, cTSelf16, base):
                """rowsrc: () -> (1,512) raw matching row (unnormalized). Emits
                the s + 16 multi cols at out12[:, base:base+17]."""
                u = f"rm{base}"
                wT = wsqT17[wname]
                rowvec = rowsrc()
                # rowvec chunks as columns (128, 4)
                psL = xt(name="psL")
                for k in range(NCH):
                    nc.tensor.matmul(psL[:, k:k + 1], lhsT=rowvec[:, CH(k)],
                                     rhs=one11[:], start=True, stop=True,
                                     skip_group_check=True)
                yield
                lcol = sb.tile([128, NCH], f32, tag=f"{u}_lcol", name="rmlcol")
                nc.vector.tensor_copy(lcol[:], psL[:, 0:NCH])
                yield
                lsq = sb.tile([128, NCH], f16, tag=f"{u}_lsq", name="rmlsq")
                nc.scalar.square(lsq[:], lcol[:])
                # w2l = wsqT17 * lcol (per chunk; ones col picks up lcol)
                w2l = sb.tile([128, 68], f16, tag=f"{u}_w2l", name="rmw2l")
                for k in range(NCH):
                    nc.gpsimd.tensor_scalar(
                        w2l[:, C17(k)], wT[:, C17(k)],
                        lcol[:, k:k + 1], None, ALU.mult)
                yield
                # one shared PSUM tile: num [.,0:17], den [0:17,17:18],
                # drow [0:1,18:35], dbc [:,35:52]
                rps = xt(name="rm_ps")
                for k in range(NCH):
                    nc.tensor.matmul(rps[:, 0:P17], lhsT=cTSelf16[:, CH(k)],
                                     rhs=w2l[:, C17(k)],
                                     start=(k == 0), stop=(k == NCH - 1))
                for k in range(NCH):
                    nc.tensor.matmul(rps[0:P17, 17:18],
                                     lhsT=wT[:, C17(k)],
                                     rhs=lsq[:, k:k + 1],
                                     start=(k == 0), stop=(k == NCH - 1),
                                     skip_group_check=True)
                yield
                dsq = sb.tile([P17, 1], f32, tag=f"{u}_dsq", name="rmdsq")
                nc.scalar.activation(dsq[:], rps[0:P17, 17:18], AFT.Sqrt,
                                     bias=epsb[0:P17, :])
                yield
                dr = sb.tile([P17, 1], f32, tag=f"{u}_dr", name="rmdr")
                nc.vector.reciprocal(dr[:], dsq[:])
                yield
                # transpose (17,1) -> (1,17), broadcast to (128,17)
                nc.tensor.matmul(rps[0:1, 18:18 + P17], lhsT=dr[:],
                                 rhs=idn[0:P17, 0:P17],
                                 start=True, stop=True, skip_group_check=True)
                yield
                drow = sb.tile([1, P17], f32, tag=f"{u}_drow", name="rmdrow")
                nc.vector.tensor_copy(drow[:], rps[0:1, 18:18 + P17])
                yield
                nc.tensor.matmul(rps[:, 35:35 + P17], lhsT=ones1[:], rhs=drow[:],
                                 start=True, stop=True, skip_group_check=True)
                yield
                t = sb.tile([128, P17], f32, tag=f"{u}_t", name="rmt")
                nc.vector.tensor_tensor(t[:], rps[:, 0:P17],
                                        rnp17[side][wname][:], ALU.mult)
                nc.vector.tensor_tensor(out12[:, base:base + P17], t[:],
                                        rps[:, 35:35 + P17], ALU.mult)

            # ---------------- attentive mean (unnormalized softmax) ---------
            def att_exp(lhsT_cos, rhs_c, mcol_, offcol, tag, store):
                s_ps = xt(name=f"sps_{tag}")
                nc.tensor.matmul(s_ps[:], lhsT=lhsT_cos[:], rhs=rhs_c[:],
                                 start=True, stop=True)
                yield
                e = sb.tile([128, 512], f32, tag=f"e_{tag}", name="esm")
                nc.scalar.activation(e[:], s_ps[:], AFT.Exp,
                                     scale=mcol_[:], bias=offcol[:])
                store(e)

            # ---------------- vector matching (v per row) ----------------
            def vec_match(vsrc, wname, side, cTSelf16, base, tag,
                          vt_act=False):
                wT = wsqT17[wname]
                v = vsrc() if callable(vsrc) else vsrc
                # vT (fp16) + vsqT (fp16)
                psT = xt(name=f"vmT_{tag}")
                for k in range(NCH):
                    nc.tensor.transpose(psT[:, CH(k)], v[:, CH(k)], idn[:])
                yield
                vT = sc.tile([128, 512], f16, tag="vm_vT", bufs=2, name="vmvT")
                if vt_act:
                    nc.scalar.copy(vT[:], psT[:])
                else:
                    nc.vector.tensor_copy(vT[:], psT[:])
                yield
                vsqT = sc.tile([128, 512], f16, tag="vm_vsqT", bufs=2,
                               name="vmvsqT")
                nc.scalar.square(vsqT[:], vT[:])
                prodT = sc.tile([128, 512], f16, tag="vm_prodT", bufs=2,
                                name="vmprodT")
                nc.vector.tensor_tensor(prodT[:], cTSelf16[:], vT[:], ALU.mult)
                yield
                nd_ps = xt(name="vm_nd")
                for k in range(NCH):
                    nc.tensor.matmul(nd_ps[:, 0:P17], lhsT=prodT[:, CH(k)],
                                     rhs=wT[:, C17(k)],
                                     start=(k == 0), stop=(k == NCH - 1))
                for k in range(NCH):
                    nc.tensor.matmul(nd_ps[:, P17:2 * P17], lhsT=vsqT[:, CH(k)],
                                     rhs=wT[:, C17(k)],
                                     start=(k == 0), stop=(k == NCH - 1),
                                     skip_group_check=True)
                yield
                dsq = sb.tile([128, P17], f32, tag=f"vm_dsq_{tag}", name="vmdsq")
                nc.scalar.activation(dsq[:], nd_ps[:, P17:2 * P17], AFT.Sqrt,
                                     bias=epsb[:])
                yield
                dr = sb.tile([128, P17], f32, tag=f"vm_dr_{tag}", name="vmdr")
                nc.vector.reciprocal(dr[:], dsq[:])
                yield
                t = sb.tile([128, P17], f32, tag=f"vm_t_{tag}", name="vmt")
                nc.vector.tensor_tensor(t[:], nd_ps[:, 0:P17],
                                        rnp17[side][wname][:], ALU.mult)
                nc.vector.tensor_tensor(out12[:, base:base + P17], t[:], dr[:],
                                        ALU.mult)

            # full-matching row extraction
            state = {}

            def do_extracts():
                oh2 = onehot_last(m2row, "2")
                oh1 = onehot_last(m1row, "1")
                yield
                oh2c = row_to_col(oh2)
                yield
                oh1c = row_to_col(oh1)
                yield
                state["c2last"] = extract_row(oh2c, ctx2, "c2l")
                yield
                state["c1last"] = extract_row(oh1c, ctx1, "c1l")

            # ================= interleaved schedule =================
            # Per side 64 product tiles; each tick: PE produces tile t for
            # both sides, consumers handle tile t-1 (one tick of slack for
            # every cross-engine dependency), and every active phase-1 task
            # generator advances exactly one stage.
            NT = 64  # tiles per side

            starters = {}  # tick -> list of generator factories

            def at_tick(t, g):
                starters.setdefault(t, []).append(g)

            # weights prep early (mp first: needed by mp_iter)
            at_tick(0, prep_w("mp"))
            at_tick(0, cos_features())
            at_tick(2, prep_rnp("mp", "1"))
            at_tick(2, prep_rnp("mp", "2"))
            at_tick(1, prep_w("ff"))
            at_tick(3, prep_rnp("ff", "1"))
            at_tick(3, prep_rnp("ff", "2"))
            at_tick(2, prep_w("bw"))
            at_tick(4, prep_rnp("bw", "1"))
            at_tick(4, prep_rnp("bw", "2"))
            at_tick(3, prep_w("at"))
            at_tick(5, prep_rnp("at", "1"))
            at_tick(5, prep_rnp("at", "2"))
            at_tick(4, prep_w("ma"))
            at_tick(6, prep_rnp("ma", "1"))
            at_tick(6, prep_rnp("ma", "2"))

            at_tick(0, do_extracts())

            # maxpool: one p every 3 ticks once rnp["mp"] is ready
            for p in range(P):
                at_tick(8 + 2 * p, mp_iter(p))

            # full matches (need rnp of their weight + extracted rows)
            at_tick(7, row_match(lambda: state["c2last"], "ff", "1", c1TL, 2))
            at_tick(10, row_match(lambda: ctx2[0:1, :], "bw", "1", c1TL, 19))
            at_tick(13, row_match(lambda: state["c1last"], "ff", "2", c2TL,
                                  102 + 2))
            at_tick(16, row_match(lambda: ctx1[0:1, :], "bw", "2", c2TL,
                                  102 + 19))

            # attentive mean (exp) + matches
            at_tick(5, att_exp(cosT, ctx2, m1col, offm1col, "2",
                               lambda e: state.__setitem__("e2", e)))
            at_tick(7, att_exp(cos, ctx1, m2col, offm2col, "1",
                               lambda e: state.__setitem__("e1", e)))
            def fixup_task():
                yield
                mp_fixups()

            at_tick(46, fixup_task())
            at_tick(40, vec_match(lambda: state["e2"], "at", "1", c1TL, 68, "a1"))
            at_tick(52, vec_match(lambda: state["e1"], "at", "2", c2TL,
                                  102 + 68, "a2"))

            NQ = NT // 2  # broadcast quads per side
            stgs = {}
            active = []
            t = 0
            while True:
                # one broadcast DMA per tick: side 2 on even, side 1 on odd
                if t < 2 * NQ:
                    side_p = "2" if t % 2 == 0 else "1"
                    stgs[(side_p, t // 2)] = loop_produce(side_p, t // 2)
                # consume the quad staged 2 ticks ago
                cq = t - 2
                if 0 <= cq < 2 * NQ:
                    side_c = "2" if cq % 2 == 0 else "1"
                    loop_consume(side_c, cq // 2, stgs.pop((side_c, cq // 2)))
                # advance tasks one stage
                for g in starters.pop(t, ()):
                    active.append(g)
                still = []
                for g in active:
                    try:
                        next(g)
                        still.append(g)
                    except StopIteration:
                        pass
                active = still
                t += 1
                if t >= 2 * NQ + 2 and not active and not starters:
                    break
                if t > 2 * NQ + 80:
                    raise RuntimeError("schedule failed to drain")

            # tails: merge + max-att matches (interleave the two chains)
            ax2 = loop_finish("2")
            ax1 = loop_finish("1")
            gens = [vec_match(ax2, "ma", "1", c1TL, 85, "x1"),
                    vec_match(ax1, "ma", "2", c2TL, 102 + 85, "x2")]
            while gens:
                nxt2 = []
                for g in gens:
                    try:
                        next(g)
                        nxt2.append(g)
                    except StopIteration:
                        pass
                gens = nxt2

            # ---------------- output ----------------
            nc.sync.dma_start(out_d[:], out12[:])

    _split_multi_waits(nc)
    return nc


_CACHE = {}


def _get_nc():
    if "nc" not in _CACHE:
        nc = bass.Bass()
        _emit(nc)
        _CACHE["nc"] = nc
    return _CACHE["nc"]


_IDN = np.eye(128, dtype=np.float32)


def run_sharded(inputs, trace=False):
    nc = _get_nc()
    in_maps = []
    for b in range(B):
        in_maps.append({
            "context_1": np.ascontiguousarray(np.asarray(inputs["context_1"][b], np.float32)),
            "mask_1": np.ascontiguousarray(np.asarray(inputs["mask_1"][b], np.float32)[None, :]),
            "context_2": np.ascontiguousarray(np.asarray(inputs["context_2"][b], np.float32)),
            "mask_2": np.ascontiguousarray(np.asarray(inputs["mask_2"][b], np.float32)[None, :]),
            "w_full_fwd": np.ascontiguousarray(np.asarray(inputs["w_full_fwd"], np.float32)),
            "w_full_bwd": np.ascontiguousarray(np.asarray(inputs["w_full_bwd"], np.float32)),
            "w_maxpool": np.ascontiguousarray(np.asarray(inputs["w_maxpool"], np.float32)),
            "w_att": np.ascontiguousarray(np.asarray(inputs["w_att"], np.float32)),
            "w_max_att": np.ascontiguousarray(np.asarray(inputs["w_max_att"], np.float32)),
            "idn": _IDN,
        })
    res = run_bass_kernel_spmd(nc, in_maps, core_ids=list(range(B)), trace=trace)
    out = np.stack([res.results[b]["out"] for b in range(B)], axis=0)
    return out, res


def kernel(context_1, mask_1, context_2, mask_2,
           w_full_fwd, w_full_bwd, w_maxpool, w_att, w_max_att):
    out, _ = run_sharded({
        "context_1": context_1, "mask_1": mask_1,
        "context_2": context_2, "mask_2": mask_2,
        "w_full_fwd": w_full_fwd, "w_full_bwd": w_full_bwd,
        "w_maxpool": w_maxpool, "w_att": w_att, "w_max_att": w_max_att,
    })
    return out


# revision 106
# speedup vs baseline: 1.0011x; 1.0011x over previous
"""BiMPM matching kernel for Trainium2 (Bass/Tile), 8-core data-parallel.

Strategy: batch B=8 is sharded one element per NeuronCore. Each core runs the
full BiMPM forward for its (L=128, D=512) pair of contexts.

v3 design (TimelineSim ~132.6us vs the 229.2us select-broadcast baseline):
  - The dominant attentive-max tensors (max_j cos[i,j]*c[j,d], both sides)
    are computed without PE or PSUM: the masked fp16 contexts round-trip
    through scratch DRAM once, then one DMA per 4-row "quad" re-reads a row
    with a zero-stride (broadcast) source AP, replicating it across all 128
    partitions straight into SBUF. The per-k cosine multiply is routed per
    quad to whichever engine has headroom - ACT (scaled copy), Pool/GPSIMD
    (tensor_scalar; the only tensor ops walrus accepts on that engine), or
    DVE itself at 4x fp16 for the early quads while the DMA ring warms up.
    DVE then max-accumulates (128,2048) fp16 tiles at 2x on two interleaved
    chains per side. This keeps DVE, ACT, Pool, and the DMA engines all
    ~95-100% busy through the body of the kernel.
  - All "single + 16 multi-perspective" cosine feature blocks use a 17-wide
    weight layout (leading ones column) so one matmul chain yields the s and
    m features contiguously, including the self-norm via the ones column.
  - Softmax normalization for attentive-mean is dropped entirely: cosine
    matching is scale-invariant per row, so a raw masked exp() suffices.
  - Maxpool / full-match / attentive-match work is decomposed into
    generator "tasks" advanced one pipeline stage per scheduler tick, so
    every engine's in-order stream stays dependency-ready.

Self-contained: hardcodes shapes B=8, L1=L2=128, D=512, P=16.
"""

import numpy as np

import concourse.bass as bass
import concourse.mybir as mybir
import concourse.tile as tile
from concourse.bass_utils import run_bass_kernel_spmd
from concourse.vector_clock import ScopedClock

f32 = mybir.dt.float32
f32r = mybir.dt.float32r
f16 = mybir.dt.float16
ALU = mybir.AluOpType
AFT = mybir.ActivationFunctionType
AX = mybir.AxisListType

B, L, D, P = 8, 128, 512, 16
NCH = D // 128  # 4 d-chunks
P17 = P + 1
NEG = -1.0e30
EPS_CNT = 1.0e-8  # matches reference EPS for count clamping
EPS_N = 1.0e-6    # per-factor norm clamp (product >= 1e-12 never binds here)
OFFBIG = 60000.0  # fp16-finite sentinel for attentive-max masking

# ---------------------------------------------------------------------------
# Workarounds: this walrus build accepts only ONE sync-wait per instruction.
# ---------------------------------------------------------------------------

def _drain_and_barrier_split(self, tick_clock, wait_clock):
    drain_inst = self.nc.sync.drain()
    wait_clock.add_sem_waits(
        drain_inst.ins, ScopedClock({None: tick_clock.global_clock})
    )
    si = drain_inst.ins.sync_info
    if si is not None and si.on_wait and len(si.on_wait) > 1:
        extra = list(si.on_wait[1:])
        del si.on_wait[1:]
        for w in extra:
            d2 = self.nc.sync.drain()
            if d2.ins.sync_info is None:
                d2.ins.sync_info = mybir.SyncInfo(on_wait=[], on_update=[])
            d2.ins.sync_info.on_wait.append(w)
    self.nc.all_engine_barrier()
    assert self.sems is not None
    popped = self.nc._tile_sem_poison_stack.pop()
    assert popped is self._sem_poison
    self.nc.clear_and_free_semaphores(list(self.sems.allocated().values()))


tile.TileContext._drain_and_barrier = _drain_and_barrier_split


def _split_multi_waits(nc):
    """Hoist extra sync-waits onto injected same-engine Drains placed before
    the owning instruction (serial on one engine == wait-all)."""
    n = 0
    for fn in nc.m.functions:
        for blk in fn.blocks:
            new = []
            for ins in blk.instructions:
                si = ins.sync_info
                if si is not None and si.on_wait and len(si.on_wait) > 1:
                    extra = list(si.on_wait[:-1])
                    keep = [si.on_wait[-1]]
                    for w in extra:
                        new.append(
                            mybir.InstDrain(
                                name=f"waitsplit-{n}",
                                engine=ins.engine,
                                is_reset_sema=False,
                                sync_info=mybir.SyncInfo(on_wait=[w], on_update=[]),
                            )
                        )
                        n += 1
                    si.on_wait = keep
                new.append(ins)
            blk.instructions = new
    return n


# ---------------------------------------------------------------------------
# Kernel emission
# ---------------------------------------------------------------------------

def CH(k):
    return slice(k * 128, (k + 1) * 128)


def C17(k):
    return slice(k * P17, (k + 1) * P17)


def _emit(nc: bass.Bass):
    ctx1_d = nc.dram_tensor("context_1", [L, D], f32, kind="ExternalInput")
    m1_d = nc.dram_tensor("mask_1", [1, L], f32, kind="ExternalInput")
    ctx2_d = nc.dram_tensor("context_2", [L, D], f32, kind="ExternalInput")
    m2_d = nc.dram_tensor("mask_2", [1, L], f32, kind="ExternalInput")
    wff_d = nc.dram_tensor("w_full_fwd", [P, D], f32, kind="ExternalInput")
    wbw_d = nc.dram_tensor("w_full_bwd", [P, D], f32, kind="ExternalInput")
    wmp_d = nc.dram_tensor("w_maxpool", [P, D], f32, kind="ExternalInput")
    wat_d = nc.dram_tensor("w_att", [P, D], f32, kind="ExternalInput")
    wma_d = nc.dram_tensor("w_max_att", [P, D], f32, kind="ExternalInput")
    idn_d = nc.dram_tensor("idn", [128, 128], f32, kind="ExternalInput")
    out_d = nc.dram_tensor("out", [L, 204], f32, kind="ExternalOutput")

    c1M_d = nc.dram_tensor("c1M_scr", [L, D], f16, kind="Internal")
    c2M_d = nc.dram_tensor("c2M_scr", [L, D], f16, kind="Internal")

    with tile.TileContext(nc) as tc:
        with tc.tile_pool(name="sb", bufs=1) as sb, \
             tc.tile_pool(name="sc", bufs=2) as sc, \
             tc.tile_pool(name="psX", bufs=6, space="PSUM") as psX:

            def xt(shape=None, name="x"):
                return psX.tile(shape or [128, 512], f32, tag="x", name=name,
                                padded_shape=[128, 512])

            def scr512():
                return sc.tile([128, 512], f32, tag="scr512", name="scr512")

            # ---------------- constants + inputs ----------------
            idn = sb.tile([128, 128], f32, tag="idn")
            nc.sync.dma_start(idn[:], idn_d[:])
            ones1 = sb.tile([1, 128], f32, tag="ones1")
            nc.vector.memset(ones1[:], 1.0)
            one11 = sb.tile([1, 1], f32, tag="one11")
            nc.vector.memset(one11[:], 1.0)
            # sqrt-bias clamp: sqrt(x + 1e-12) == max(sqrt(x), 1e-6)
            epsb = sb.tile([128, 1], f32, tag="epsb")
            nc.vector.memset(epsb[:], 1.0e-12)

            ctx1 = sb.tile([128, 512], f32, tag="ctx1")
            nc.sync.dma_start(ctx1[:], ctx1_d[:])
            ctx2 = sb.tile([128, 512], f32, tag="ctx2")
            nc.sync.dma_start(ctx2[:], ctx2_d[:])
            m1row = sb.tile([1, 128], f32, tag="m1row")
            nc.sync.dma_start(m1row[:], m1_d[:])
            m2row = sb.tile([1, 128], f32, tag="m2row")
            nc.sync.dma_start(m2row[:], m2_d[:])
            wdr = {}
            for wname, wd in (("ff", wff_d), ("bw", wbw_d), ("mp", wmp_d),
                              ("at", wat_d), ("ma", wma_d)):
                wt = sb.tile([P, 512], f32, tag=f"w_{wname}", name=f"w_{wname}")
                nc.sync.dma_start(wt[:], wd[:])
                wdr[wname] = wt

            out12 = sb.tile([128, 204], f32, tag="out12")

            # ---------------- masks / columns ----------------
            def row_to_col(row, n=128):
                ps = xt(name="r2c")
                nc.tensor.matmul(ps[:n, 0:1], lhsT=row[:, 0:n], rhs=one11[:],
                                 start=True, stop=True)
                col = sb.tile([n, 1], f32, tag=f"col{nc.next_id()}", name="col")
                nc.vector.tensor_copy(col[:], ps[:n, 0:1])
                return col

            m1col = row_to_col(m1row)
            m2col = row_to_col(m2row)

            def ts_new(shape, tag, in0, s1, s2, op0, op1=None):
                t = sb.tile(shape, f32, tag=tag, name=tag)
                if op1 is None:
                    nc.vector.tensor_scalar(t[:], in0[:], s1, None, op0)
                else:
                    nc.vector.tensor_scalar(t[:], in0[:], s1, s2, op0, op1)
                return t

            # softmax bias (-1e30 at invalid rows, f32 domain)
            offm1col = ts_new([128, 1], "offm1col", m1col, -1.0, 1.0e30, ALU.add, ALU.mult)
            offm2col = ts_new([128, 1], "offm2col", m2col, -1.0, 1.0e30, ALU.add, ALU.mult)
            # att-max sentinels (fp16-finite)
            offb1col = ts_new([128, 1], "offb1col", m1col, -1.0, OFFBIG, ALU.add, ALU.mult)
            offb2col = ts_new([128, 1], "offb2col", m2col, -1.0, OFFBIG, ALU.add, ALU.mult)
            # +1 at invalid columns (for the cosM shift)
            invm1row = ts_new([1, 128], "invm1row", m1row, -1.0, 1.0, ALU.mult, ALU.add)
            invm2row = ts_new([1, 128], "invm2row", m2row, -1.0, 1.0, ALU.mult, ALU.add)
            # -1e30 at invalid columns (maxpool fills, f32 domain)
            offm1row = ts_new([1, 128], "offm1row", m1row, -1.0, 1.0e30, ALU.add, ALU.mult)
            offm2row = ts_new([1, 128], "offm2row", m2row, -1.0, 1.0e30, ALU.add, ALU.mult)

            # counts: rcnt = 1/max(sum(mask), EPS)
            def rcnt_of(mrow, tag):
                s = sb.tile([1, 1], f32, tag=f"cnt_{tag}", name="cnt")
                nc.vector.tensor_reduce(s[:], mrow[:], AX.X, ALU.add)
                sc_ = sb.tile([1, 1], f32, tag=f"cntc_{tag}", name="cntc")
                nc.vector.tensor_scalar(sc_[:], s[:], EPS_CNT, None, ALU.max)
                r = sb.tile([1, 1], f32, tag=f"rcnt_{tag}", name="rcnt")
                nc.vector.reciprocal(r[:], sc_[:])
                return r

            rcnt1 = rcnt_of(m1row, "1")
            rcnt2 = rcnt_of(m2row, "2")
            m1rowS = ts_new([1, 128], "m1rowS", m1row, rcnt1[:], None, ALU.mult)
            m2rowS = ts_new([1, 128], "m2rowS", m2row, rcnt2[:], None, ALU.mult)
            m1sd = row_to_col(m1rowS)  # mask/cnt column, for PE mean-reduces
            m2sd = row_to_col(m2rowS)

            # broadcast rows across partitions (PE outer product)
            def bcast_row(row, tag, act=False):
                ps = xt(name="bcr")
                nc.tensor.matmul(ps[:, 0:128], lhsT=ones1[:], rhs=row[:],
                                 start=True, stop=True)
                t = sb.tile([128, 128], f32, tag=tag, name=tag)
                if act:
                    nc.scalar.copy(t[:], ps[:, 0:128])
                else:
                    nc.vector.tensor_copy(t[:], ps[:, 0:128])
                return t

            bcOff1 = bcast_row(offm1row, "bcOff1")
            bcOff2 = bcast_row(offm2row, "bcOff2")

            # ---------------- norms + normalized contexts ----------------
            def normalize(cx, mcol_, tag):
                nsq = sb.tile([128, 1], f32, tag=f"nsq_{tag}", name="nsq")
                nc.scalar.activation(scr512()[:], cx[:], AFT.Square, accum_out=nsq[:])
                nn_ = sb.tile([128, 1], f32, tag=f"nn_{tag}", name="nn")
                nc.scalar.activation(nn_[:], nsq[:], AFT.Sqrt, bias=epsb[:])
                rn = sb.tile([128, 1], f32, tag=f"rn_{tag}", name="rn")
                nc.vector.reciprocal(rn[:], nn_[:])
                # fold the row mask into the normalization scale
                rnm = sb.tile([128, 1], f32, tag=f"rnm_{tag}", name="rnm")
                nc.vector.tensor_tensor(rnm[:], rn[:], mcol_[:], ALU.mult)
                cn = sb.tile([128, 512], f32, tag=f"cn_{tag}", name="cn")
                nc.scalar.activation(cn[:], cx[:], AFT.Copy, scale=rnm[:])
                return cn

            cn1 = normalize(ctx1, m1col, "1")
            cn2 = normalize(ctx2, m2col, "2")

            # transposed normalized contexts: cT (f32 for cosine) + fp16 copy
            def transpose_pair(src, tag):
                ps = xt(name=f"T_{tag}")
                for k in range(NCH):
                    nc.tensor.transpose(ps[:, CH(k)], src[:, CH(k)], idn[:])
                t32 = sb.tile([128, 512], f32, tag=f"{tag}32", name=f"{tag}32")
                nc.scalar.copy(t32[:], ps[:])
                t16 = sb.tile([128, 512], f16, tag=f"{tag}16", name=f"{tag}16")
                nc.vector.tensor_copy(t16[:], ps[:])
                return t32, t16

            c1T, c1TL = transpose_pair(cn1, "c1T")
            c2T, c2TL = transpose_pair(cn2, "c2T")
            c1sqT = sb.tile([128, 512], f16, tag="c1sqT")
            nc.scalar.square(c1sqT[:], c1TL[:])
            c2sqT = sb.tile([128, 512], f16, tag="c2sqT")
            nc.scalar.square(c2sqT[:], c2TL[:])

            # masked offsets for the att-max inputs (Pool add, fp16 out),
            # then staged to scratch DRAM for the broadcast loop DMAs
            c1M = sb.tile([128, 512], f16, tag="c1M")
            nc.gpsimd.tensor_scalar(c1M[:], ctx1[:], offb1col[:], None, ALU.add)
            c2M = sb.tile([128, 512], f16, tag="c2M")
            nc.gpsimd.tensor_scalar(c2M[:], ctx2[:], offb2col[:], None, ALU.add)
            nc.sync.dma_start(c1M_d[:], c1M[:])
            nc.sync.dma_start(c2M_d[:], c2M[:])

            # ---------------- cosine ----------------
            cos_ps = xt(name="cos_ps")
            for k in range(NCH):
                nc.tensor.matmul(cos_ps[:, 0:128], lhsT=c1T[:, CH(k)],
                                 rhs=c2T[:, CH(k)],
                                 start=(k == 0), stop=(k == NCH - 1))
            cos = sb.tile([128, 128], f32, tag="cos")
            nc.vector.tensor_copy(cos[:], cos_ps[:, 0:128])
            # bake the +1-at-invalid-j shift into the PSUM, then copy (scales)
            nc.tensor.matmul(cos_ps[:, 0:128], lhsT=ones1[:], rhs=invm2row[:],
                             start=False, stop=True, skip_group_check=True)
            cosM = sb.tile([128, 128], f32, tag="cosM")
            nc.vector.tensor_copy(cosM[:], cos_ps[:, 0:128])

            cosT_ps = xt(name="cosT_ps")
            nc.tensor.transpose(cosT_ps[:, 0:128], cos[:], idn[:])
            cosT = sb.tile([128, 128], f32, tag="cosT")
            nc.vector.tensor_copy(cosT[:], cosT_ps[:, 0:128])
            nc.tensor.matmul(cosT_ps[:, 0:128], lhsT=ones1[:], rhs=invm1row[:],
                             start=False, stop=True, skip_group_check=True)
            cosMT = sb.tile([128, 128], f32, tag="cosMT")
            nc.vector.tensor_copy(cosMT[:], cosT_ps[:, 0:128])
            idnL = sb.tile([128, 128], f16, tag="idnL")
            nc.gpsimd.tensor_copy(idnL[:], idn[:])

            # ---------------- cos_max / cos_mean (out cols 0,1 / 102,103) ----
            def cos_features():
                scrs = []
                for (csrc, cTsrc, bcOff, msd, base) in (
                        (cos, cosT, bcOff2, m2sd, 0),
                        (cosT, cos, bcOff1, m1sd, 102)):
                    t = sc.tile([128, 128], f32, tag="cfscr", name="cfscr")
                    nc.vector.tensor_tensor(t[:], csrc[:], bcOff[:], ALU.add)
                    mps = xt(name="cmean")
                    nc.tensor.matmul(mps[:, 0:1], lhsT=cTsrc[:], rhs=msd[:],
                                     start=True, stop=True)
                    scrs.append((t, mps, base))
                yield
                for t, mps, base in scrs:
                    nc.vector.tensor_reduce(out12[:, base:base + 1], t[:],
                                            AX.X, ALU.max)
                    nc.vector.tensor_copy(out12[:, base + 1:base + 2],
                                          mps[:, 0:1])

            # ---------------- per-weight prep: wsqT17 + rnp17 ----------------
            # wsqT17: (128, 68) fp16; chunk k cols [17k]=ones, [17k+1..17k+16]=
            # (w^2 chunk k Transposed). rnp17: (128,17) with col0 = 1 (self
            # rows are unit-norm), cols 1..16 = 1/||w_p o cn||.
            wsqT17 = {}
            rnp17 = {"1": {}, "2": {}}

            def prep_w(wname):
                wt = wdr[wname]
                wT = sb.tile([128, 68], f16, tag=f"wsqT_{wname}", name="wsqT")
                nc.gpsimd.memset(wT[:], 1.0)
                wsq = sc.tile([P, 512], f32, tag="wsq", name="wsq", bufs=3)
                nc.scalar.square(wsq[:], wt[:])
                yield
                psW = xt(name="psW")
                for k in range(NCH):
                    nc.tensor.transpose(psW[:, 16 * k:16 * (k + 1)],
                                        wsq[:, CH(k)], idn[0:P, 0:P])
                yield
                for k in range(NCH):
                    nc.vector.tensor_copy(wT[:, 17 * k + 1:17 * (k + 1)],
                                          psW[:, 16 * k:16 * (k + 1)])
                wsqT17[wname] = wT
                if wname == "mp":
                    w32 = sb.tile([128, 64], f32, tag="wsqT32mp", name="wsqT32")
                    nc.vector.tensor_copy(w32[:], psW[:, 0:64])
                    wsqT17["mp32"] = w32

            def prep_rnp(wname, side):
                csqT = c1sqT if side == "1" else c2sqT
                ps = xt(name="psnp")
                for k in range(NCH):
                    nc.tensor.matmul(ps[:, 0:P17], lhsT=csqT[:, CH(k)],
                                     rhs=wsqT17[wname][:, C17(k)],
                                     start=(k == 0), stop=(k == NCH - 1))
                yield
                sq = sb.tile([128, P17], f32, tag=f"npsq_{wname}{side}", name="npsq")
                nc.scalar.activation(sq[:], ps[:, 0:P17], AFT.Sqrt,
                                     bias=epsb[:])
                yield
                r = sb.tile([128, P17], f32, tag=f"rnp_{wname}{side}", name="rnp")
                nc.vector.reciprocal(r[:], sq[:])
                rnp17[side][wname] = r

            # ---------------- attentive-max loop pieces ----------------
            # Per side and k-quad: one DMA broadcasts rows 4t..4t+3 of the
            # DRAM-staged cM to all 128 partitions (SBUF fp16). The per-k
            # cosine multiply runs on ACT (scaled copy) or Pool (tensor
            # scalar); DVE only max-accumulates (fp16 2x), on two chains per
            # side. No PE or PSUM in the loop.
            accB = {"2": [sb.tile([128, 4, 512], f16, tag=f"acc2{c}",
                                  name="acc") for c in (0, 1)],
                    "1": [sb.tile([128, 4, 512], f16, tag=f"acc1{c}",
                                  name="acc") for c in (0, 1)]}
            first_b = {"2": [True, True], "1": [True, True]}

            NPEQ = 0  # early quads per side routed via PE/PSUM (DMA is busy
            # with input loads then; PE is otherwise idle)

            def loop_produce(side, q):
                """Stage k = 4q..4q+3 (a 'quad'): broadcast DMA from scratch
                DRAM, or PE select-broadcast into PSUM for the early quads."""
                if q < NPEQ:
                    rhs = c2M if side == "2" else c1M
                    pss = []
                    for u in range(4):
                        ps = xt(name="peq")
                        nc.tensor.matmul(
                            ps[:],
                            lhsT=idnL[:, 4 * q + u:4 * q + u + 1]
                            .broadcast_to([128, 128]),
                            rhs=rhs[:], start=True, stop=True,
                            skip_group_check=True)
                        pss.append(ps)
                    return pss
                src_d = c2M_d if side == "2" else c1M_d
                stg = sc.tile([128, 4, 512], f16, tag="stg", bufs=8,
                              name="stg")
                nc.sync.dma_start(
                    stg[:], src_d[4 * q:4 * q + 4, :].unsqueeze(0)
                    .broadcast_to([128, 4, 512]))
                return stg

            def loop_consume(side, q, stg):
                """Consume one staged quad: 4 scaled mults + one fused max."""
                k0 = 4 * q
                csc = cosM if side == "2" else cosMT
                chain = q % 2
                pe_quad = q < NPEQ
                dve_quad = (not pe_quad) and q < (6 if side == "2" else 5)
                use_pool = (not pe_quad) and (not dve_quad) and (
                    (q % 9 in (1, 3, 5, 7)) if side == "2" else
                    (q % 9 in (0, 2, 4, 6)))
                if first_b[side][chain]:
                    dst = accB[side][chain]
                    first_b[side][chain] = False
                else:
                    dst = sc.tile([128, 4, 512], f16, tag="bch", bufs=14,
                                  name="bch")
                for u in range(4):
                    src = stg[u][:] if pe_quad else stg[:, u, :]
                    if use_pool:
                        nc.gpsimd.tensor_scalar(
                            dst[:, u, :], src,
                            csc[:, k0 + u:k0 + u + 1], None, ALU.mult)
                    elif dve_quad and u < 2:
                        nc.vector.tensor_scalar(
                            dst[:, u, :], src,
                            csc[:, k0 + u:k0 + u + 1], None, ALU.mult)
                    else:
                        nc.scalar.activation(
                            dst[:, u, :], src, AFT.Copy,
                            scale=csc[:, k0 + u:k0 + u + 1])
                if dst is not accB[side][chain]:
                    nc.vector.tensor_tensor(accB[side][chain][:], dst[:],
                                            accB[side][chain][:], ALU.max)

            def loop_finish(side):
                m1 = sb.tile([128, 4, 512], f16, tag=f"axm_{side}", name="axm")
                nc.vector.tensor_tensor(m1[:], accB[side][0][:],
                                        accB[side][1][:], ALU.max)
                m2 = sb.tile([128, 2, 512], f16, tag=f"axn_{side}", name="axn")
                nc.vector.tensor_tensor(m2[:], m1[:, 0:2, :], m1[:, 2:4, :],
                                        ALU.max)
                ax = sb.tile([128, 512], f32, tag=f"ax_{side}", name="ax")
                nc.vector.tensor_tensor(ax[:], m2[:, 0, :], m2[:, 1, :],
                                        ALU.max)
                return ax

            # ---------------- maxpool matching ----------------
            def mp_iter(p):
                rnp1mp = rnp17["1"]["mp"]
                rnp2mp = rnp17["2"]["mp"]
                w32 = wsqT17["mp32"]
                wc = sc.tile([128, 512], f16, tag="wc", bufs=3, name="wc")
                for k in range(NCH):
                    nc.vector.tensor_scalar(
                        wc[:, CH(k)], c1TL[:, CH(k)],
                        w32[:, 16 * k + p:16 * k + p + 1], None, ALU.mult)
                yield
                mp_ps = xt(name="mp_ps")
                for k in range(NCH):
                    nc.tensor.matmul(mp_ps[:, 0:128], lhsT=wc[:, CH(k)],
                                     rhs=c2TL[:, CH(k)],
                                     start=(k == 0), stop=(k == NCH - 1))
                yield
                t1 = sc.tile([128, 128], f32, tag="mv_t1", bufs=3, name="mv_t1")
                if p % 2 == 0:
                    nc.scalar.activation(t1[:], mp_ps[:, 0:128], AFT.Copy,
                                         scale=rnp1mp[:, 1 + p:2 + p])
                else:
                    nc.vector.tensor_scalar(t1[:], mp_ps[:, 0:128],
                                            rnp1mp[:, 1 + p:2 + p], None,
                                            ALU.mult)
                yield
                t1T_ps = xt(name="t1T")
                nc.tensor.transpose(t1T_ps[:, 0:128], t1[:], idn[:])
                # fold the mask-1 fill (along free i) in via a PE accumulate
                nc.tensor.matmul(t1T_ps[:, 0:128], lhsT=ones1[:], rhs=offm1row[:],
                                 start=False, stop=True, skip_group_check=True)
                yield
                npt = sc.tile([128, 128], f32, tag="mv_npt", bufs=3, name="mv_npt")
                if p % 2 == 1:
                    nc.scalar.activation(npt[:], t1T_ps[:, 0:128], AFT.Copy,
                                         scale=rnp2mp[:, 1 + p:2 + p])
                else:
                    nc.vector.tensor_scalar(npt[:], t1T_ps[:, 0:128],
                                            rnp2mp[:, 1 + p:2 + p], None,
                                            ALU.mult)
                yield
                np_ps = xt(name="npT")
                nc.tensor.transpose(np_ps[:, 0:128], npt[:], idn[:])
                nc.tensor.matmul(np_ps[:, 0:128], lhsT=ones1[:], rhs=offm2row[:],
                                 start=False, stop=True, skip_group_check=True)
                # masked means as PE reductions against mask/cnt columns,
                # sharing the np_ps PSUM tile (cols 128,129)
                nc.tensor.matmul(np_ps[:, 128:129], lhsT=npt[:], rhs=m2sd[:],
                                 start=True, stop=True, skip_group_check=True)
                nc.tensor.matmul(np_ps[:, 129:130], lhsT=t1[:], rhs=m1sd[:],
                                 start=True, stop=True, skip_group_check=True)
                yield
                # (i,j) orientation (np_ps, PSUM) reduces over j; (j,i) over i
                nc.vector.tensor_reduce(out12[:, 36 + p:37 + p],
                                        np_ps[:, 0:128], AX.X, ALU.max)
                nc.vector.tensor_reduce(out12[:, 102 + 36 + p:102 + 37 + p],
                                        npt[:], AX.X, ALU.max)
                nc.vector.tensor_copy(out12[:, 52 + p:53 + p], np_ps[:, 128:129])
                nc.vector.tensor_scalar(out12[:, 102 + 52 + p:102 + 53 + p],
                                        np_ps[:, 129:130], rnp2mp[:, 1 + p:2 + p],
                                        None, ALU.mult)

            def mp_fixups():
                # invalid-i rows of the mv1 blocks picked up the transposed
                # mask-1 fill term; reference value there is exactly 0, and
                # (-huge) * 0 == -0, so a mask multiply restores it.
                nc.gpsimd.tensor_scalar(out12[:, 36:68], out12[:, 36:68],
                                        m1col[:], None, ALU.mult)

            # ---------------- full matching (last/first rows) ----------------
            def onehot_last(mrow, tag):
                oh = sb.tile([1, 128], f32, tag=f"oh_{tag}", name="oh")
                nc.vector.tensor_sub(oh[:, 0:127], mrow[:, 0:127], mrow[:, 1:128])
                nc.vector.tensor_copy(oh[:, 127:128], mrow[:, 127:128])
                return oh

            def extract_row(coltile, src, tag):
                ps = xt(name="exr")
                nc.tensor.matmul(ps[0:1, :], lhsT=coltile[:], rhs=src[:],
                                 start=True, stop=True)
                t = sb.tile([1, 512], f32, tag=f"row_{tag}", name="rowx")
                nc.vector.tensor_copy(t[:], ps[0:1, :])
                return t

            def row_match(rowsrc, wname, side, cTSelf16, base):
                """rowsrc: () -> (1,512) raw matching row (unnormalized). Emits
                the s + 16 multi cols at out12[:, base:base+17]."""
                u = f"rm{base}"
                wT = wsqT17[wname]
                rowvec = rowsrc()
                # rowvec chunks as columns (128, 4)
                psL = xt(name="psL")
                for k in range(NCH):
                    nc.tensor.matmul(psL[:, k:k + 1], lhsT=rowvec[:, CH(k)],
                                     rhs=one11[:], start=True, stop=True,
                                     skip_group_check=True)
                yield
                lcol = sb.tile([128, NCH], f32, tag=f"{u}_lcol", name="rmlcol")
                nc.vector.tensor_copy(lcol[:], psL[:, 0:NCH])
                yield
                lsq = sb.tile([128, NCH], f16, tag=f"{u}_lsq", name="rmlsq")
                nc.scalar.square(lsq[:], lcol[:])
                # w2l = wsqT17 * lcol (per chunk; ones col picks up lcol)
                w2l = sb.tile([128, 68], f16, tag=f"{u}_w2l", name="rmw2l")
                for k in range(NCH):
                    nc.gpsimd.tensor_scalar(
                        w2l[:, C17(k)], wT[:, C17(k)],
                        lcol[:, k:k + 1], None, ALU.mult)
                yield
                # one shared PSUM tile: num [.,0:17], den [0:17,17:18],
                # drow [0:1,18:35], dbc [:,35:52]
                rps = xt(name="rm_ps")
                for k in range(NCH):
                    nc.tensor.matmul(rps[:, 0:P17], lhsT=cTSelf16[:, CH(k)],
                                     rhs=w2l[:, C17(k)],
                                     start=(k == 0), stop=(k == NCH - 1))
                for k in range(NCH):
                    nc.tensor.matmul(rps[0:P17, 17:18],
                                     lhsT=wT[:, C17(k)],
                                     rhs=lsq[:, k:k + 1],
                                     start=(k == 0), stop=(k == NCH - 1),
                                     skip_group_check=True)
                yield
                dsq = sb.tile([P17, 1], f32, tag=f"{u}_dsq", name="rmdsq")
                nc.scalar.activation(dsq[:], rps[0:P17, 17:18], AFT.Sqrt,
                                     bias=epsb[0:P17, :])
                yield
                dr = sb.tile([P17, 1], f32, tag=f"{u}_dr", name="rmdr")
                nc.vector.reciprocal(dr[:], dsq[:])
                yield
                # transpose (17,1) -> (1,17), broadcast to (128,17)
                nc.tensor.matmul(rps[0:1, 18:18 + P17], lhsT=dr[:],
                                 rhs=idn[0:P17, 0:P17],
                                 start=True, stop=True, skip_group_check=True)
                yield
                drow = sb.tile([1, P17], f32, tag=f"{u}_drow", name="rmdrow")
                nc.vector.tensor_copy(drow[:], rps[0:1, 18:18 + P17])
                yield
                nc.tensor.matmul(rps[:, 35:35 + P17], lhsT=ones1[:], rhs=drow[:],
                                 start=True, stop=True, skip_group_check=True)
                yield
                t = sb.tile([128, P17], f32, tag=f"{u}_t", name="rmt")
                nc.vector.tensor_tensor(t[:], rps[:, 0:P17],
                                        rnp17[side][wname][:], ALU.mult)
                nc.vector.tensor_tensor(out12[:, base:base + P17], t[:],
                                        rps[:, 35:35 + P17], ALU.mult)

            # ---------------- attentive mean (unnormalized softmax) ---------
            def att_exp(lhsT_cos, rhs_c, mcol_, offcol, tag, store):
                s_ps = xt(name=f"sps_{tag}")
                nc.tensor.matmul(s_ps[:], lhsT=lhsT_cos[:], rhs=rhs_c[:],
                                 start=True, stop=True)
                yield
                e = sb.tile([128, 512], f32, tag=f"e_{tag}", name="esm")
                nc.scalar.activation(e[:], s_ps[:], AFT.Exp,
                                     scale=mcol_[:], bias=offcol[:])
                store(e)

            # ---------------- vector matching (v per row) ----------------
            def vec_match(vsrc, wname, side, cTSelf16, base, tag,
                          vt_act=False):
                wT = wsqT17[wname]
                v = vsrc() if callable(vsrc) else vsrc
                # vT (fp16) + vsqT (fp16)
                psT = xt(name=f"vmT_{tag}")
                for k in range(NCH):
                    nc.tensor.transpose(psT[:, CH(k)], v[:, CH(k)], idn[:])
                yield
                vT = sc.tile([128, 512], f16, tag="vm_vT", bufs=2, name="vmvT")
                if vt_act:
                    nc.scalar.copy(vT[:], psT[:])
                else:
                    nc.vector.tensor_copy(vT[:], psT[:])
                yield
                vsqT = sc.tile([128, 512], f16, tag="vm_vsqT", bufs=2,
                               name="vmvsqT")
                nc.scalar.square(vsqT[:], vT[:])
                prodT = sc.tile([128, 512], f16, tag="vm_prodT", bufs=2,
                                name="vmprodT")
                nc.vector.tensor_tensor(prodT[:], cTSelf16[:], vT[:], ALU.mult)
                yield
                nd_ps = xt(name="vm_nd")
                for k in range(NCH):
                    nc.tensor.matmul(nd_ps[:, 0:P17], lhsT=prodT[:, CH(k)],
                                     rhs=wT[:, C17(k)],
                                     start=(k == 0), stop=(k == NCH - 1))
                for k in range(NCH):
                    nc.tensor.matmul(nd_ps[:, P17:2 * P17], lhsT=vsqT[:, CH(k)],
                                     rhs=wT[:, C17(k)],
                                     start=(k == 0), stop=(k == NCH - 1),
                                     skip_group_check=True)
                yield
                dsq = sb.tile([128, P17], f32, tag=f"vm_dsq_{tag}", name="vmdsq")
                nc.scalar.activation(dsq[:], nd_ps[:, P17:2 * P17], AFT.Sqrt,
                                     bias=epsb[:])
                yield
                dr = sb.tile([128, P17], f32, tag=f"vm_dr_{tag}", name="vmdr")
                nc.vector.reciprocal(dr[:], dsq[:])
                yield
                t = sb.tile([128, P17], f32, tag=f"vm_t_{tag}", name="vmt")
                nc.vector.tensor_tensor(t[:], nd_ps[:, 0:P17],
                                        rnp17[side][wname][:], ALU.mult)
                nc.vector.tensor_tensor(out12[:, base:base + P17], t[:], dr[:],
                                        ALU.mult)

            # full-matching row extraction
            state = {}

            def do_extracts():
                oh2 = onehot_last(m2row, "2")
                oh1 = onehot_last(m1row, "1")
                yield
                oh2c = row_to_col(oh2)
                yield
                oh1c = row_to_col(oh1)
                yield
                state["c2last"] = extract_row(oh2c, ctx2, "c2l")
                yield
                state["c1last"] = extract_row(oh1c, ctx1, "c1l")

            # ================= interleaved schedule =================
            # Per side 64 product tiles; each tick: PE produces tile t for
            # both sides, consumers handle tile t-1 (one tick of slack for
            # every cross-engine dependency), and every active phase-1 task
            # generator advances exactly one stage.
            NT = 64  # tiles per side

            starters = {}  # tick -> list of generator factories

            def at_tick(t, g):
                starters.setdefault(t, []).append(g)

            # weights prep early (mp first: needed by mp_iter)
            at_tick(0, prep_w("mp"))
            at_tick(0, cos_features())
            at_tick(2, prep_rnp("mp", "1"))
            at_tick(2, prep_rnp("mp", "2"))
            at_tick(1, prep_w("ff"))
            at_tick(3, prep_rnp("ff", "1"))
            at_tick(3, prep_rnp("ff", "2"))
            at_tick(2, prep_w("bw"))
            at_tick(4, prep_rnp("bw", "1"))
            at_tick(4, prep_rnp("bw", "2"))
            at_tick(3, prep_w("at"))
            at_tick(5, prep_rnp("at", "1"))
            at_tick(5, prep_rnp("at", "2"))
            at_tick(4, prep_w("ma"))
            at_tick(6, prep_rnp("ma", "1"))
            at_tick(6, prep_rnp("ma", "2"))

            at_tick(0, do_extracts())

            # maxpool: one p every 3 ticks once rnp["mp"] is ready
            for p in range(P):
                at_tick(8 + 2 * p, mp_iter(p))

            # full matches (need rnp of their weight + extracted rows)
            at_tick(7, row_match(lambda: state["c2last"], "ff", "1", c1TL, 2))
            at_tick(10, row_match(lambda: ctx2[0:1, :], "bw", "1", c1TL, 19))
            at_tick(13, row_match(lambda: state["c1last"], "ff", "2", c2TL,
                                  102 + 2))
            at_tick(16, row_match(lambda: ctx1[0:1, :], "bw", "2", c2TL,
                                  102 + 19))

            # attentive mean (exp) + matches
            at_tick(5, att_exp(cosT, ctx2, m1col, offm1col, "2",
                               lambda e: state.__setitem__("e2", e)))
            at_tick(7, att_exp(cos, ctx1, m2col, offm2col, "1",
                               lambda e: state.__setitem__("e1", e)))
            def fixup_task():
                yield
                mp_fixups()

            at_tick(46, fixup_task())
            at_tick(40, vec_match(lambda: state["e2"], "at", "1", c1TL, 68, "a1"))
            at_tick(52, vec_match(lambda: state["e1"], "at", "2", c2TL,
                                  102 + 68, "a2"))

            NQ = NT // 2  # broadcast quads per side
            stgs = {}
            active = []
            t = 0
            while True:
                # one broadcast DMA per tick: side 2 on even, side 1 on odd
                if t < 2 * NQ:
                    side_p = "2" if t % 2 == 0 else "1"
                    stgs[(side_p, t // 2)] = loop_produce(side_p, t // 2)
                # consume the quad staged 2 ticks ago
                cq = t - 2
                if 0 <= cq < 2 * NQ:
                    side_c = "2" if cq % 2 == 0 else "1"
                    loop_consume(side_c, cq // 2, stgs.pop((side_c, cq // 2)))
                # advance tasks one stage
                for g in starters.pop(t, ()):
                    active.append(g)
                still = []
                for g in active:
                    try:
                        next(g)
                        still.append(g)
                    except StopIteration:
                        pass
                active = still
                t += 1
                if t >= 2 * NQ + 2 and not active and not starters:
                    break
                if t > 2 * NQ + 80:
                    raise RuntimeError("schedule failed to drain")

            # tails: merge + max-att matches (interleave the two chains)
            ax2 = loop_finish("2")
            ax1 = loop_finish("1")
            gens = [vec_match(ax2, "ma", "1", c1TL, 85, "x1"),
                    vec_match(ax1, "ma", "2", c2TL, 102 + 85, "x2")]
            while gens:
                nxt2 = []
                for g in gens:
                    try:
                        next(g)
                        nxt2.append(g)
                    except StopIteration:
                        pass
                gens = nxt2

            # ---------------- output ----------------
            nc.sync.dma_start(out_d[:], out12[:])

    _split_multi_waits(nc)
    return nc


_CACHE = {}


def _get_nc():
    if "nc" not in _CACHE:
        nc = bass.Bass()
        _emit(nc)
        _CACHE["nc"] = nc
    return _CACHE["nc"]


_IDN = np.eye(128, dtype=np.float32)


def run_sharded(inputs, trace=False):
    nc = _get_nc()
    in_maps = []
    for b in range(B):
        in_maps.append({
            "context_1": np.ascontiguousarray(np.asarray(inputs["context_1"][b], np.float32)),
            "mask_1": np.ascontiguousarray(np.asarray(inputs["mask_1"][b], np.float32)[None, :]),
            "context_2": np.ascontiguousarray(np.asarray(inputs["context_2"][b], np.float32)),
            "mask_2": np.ascontiguousarray(np.asarray(inputs["mask_2"][b], np.float32)[None, :]),
            "w_full_fwd": np.ascontiguousarray(np.asarray(inputs["w_full_fwd"], np.float32)),
            "w_full_bwd": np.ascontiguousarray(np.asarray(inputs["w_full_bwd"], np.float32)),
            "w_maxpool": np.ascontiguousarray(np.asarray(inputs["w_maxpool"], np.float32)),
            "w_att": np.ascontiguousarray(np.asarray(inputs["w_att"], np.float32)),
            "w_max_att": np.ascontiguousarray(np.asarray(inputs["w_max_att"], np.float32)),
            "idn": _IDN,
        })
    res = run_bass_kernel_spmd(nc, in_maps, core_ids=list(range(B)), trace=trace)
    out = np.stack([res.results[b]["out"] for b in range(B)], axis=0)
    return out, res


def kernel(context_1, mask_1, context_2, mask_2,
           w_full_fwd, w_full_bwd, w_maxpool, w_att, w_max_att):
    out, _ = run_sharded({
        "context_1": context_1, "mask_1": mask_1,
        "context_2": context_2, "mask_2": mask_2,
        "w_full_fwd": w_full_fwd, "w_full_bwd": w_full_bwd,
        "w_maxpool": w_maxpool, "w_att": w_att, "w_max_att": w_max_att,
    })
    return out
